# revision 38
# baseline (speedup 1.0000x reference)
"""Trainium2 Bass kernel for nn_DisLoss (prototype EMA + masked pairwise exp-sim loss).

Strategy (8 NeuronCores, SPMD):
  - The sequential per-sample EMA scan factors into independent per-class chains
    (order only matters within a class).  Chains are computed vectorized: lanes =
    distinct labels (sorted by chain length desc), rounds = occurrence index.
  - Chain inputs (distinct-label proto rows + per-round feature rows) are
    pre-gathered BY THE HOST into one lane-major array (identical for all
    cores), so the device does a single contiguous DMA instead of a slow
    gpsimd dma_gather descriptor storm.
  - Each core receives class-rotated copies of the prototypes so that "its" 1024
    rows are rows 0..1023; one compiled program serves all 8 cores.
  - Updated rows are cast to fp16 and scattered (indirect DMA, one call per
    2048-row quarter of the proto matrix) into a host-cast fp16 DRAM proto
    copy; protoT [256, 8192] is produced by xbar DMA transposes on the Sync
    HWDGE ring only (keeping the ACT engine free for the exp stream), each
    row-chunk depending only on its own quarter's scatter.  Each core computes
    its [1024, 8192] block of exp(P'P'^T/T) in fp16 matmuls (fp32 PSUM
    accumulate; loss rel err ~1e-7 vs fp32), with the diagonal masked to -BIG
    before the exp, ACT Exp accum_out row-sums, Ln, and an on-chip partial
    reduction.  The host sums 8 scalars.
"""

import math
from contextlib import ExitStack

import numpy as np

import types as _pytypes

import bass_rust as _bass_rust
import concourse.bass as bass
import concourse.mybir as mybir
import concourse.tile as tile
from concourse import bacc
from concourse.bass_utils import run_bass_kernel_spmd
from concourse.hw_specs import get_activation_tables
from concourse.masks import make_identity
from concourse.tile_rust import add_dep_helper

ACT_SET = "natural_log_exp_and_others"  # contains every ACT func we use


def _pin_act_tables(nc):
    """Force all activations onto one table set: the default chooser alternates
    between exp_and_others and natural_log_exp_and_others, paying ~1.3us per
    reload.  Emptying the other sets' membership (indices preserved) pins it."""

    def patched(self):
        has_act = any(
            isinstance(i, mybir.InstActivation)
            for b in self.main_func.blocks
            for i in b.instructions
        )
        if not has_act:
            return
        tables = [
            (name, fns if name == ACT_SET else type(fns)())
            for name, fns in get_activation_tables(self.m.arch).items()
        ]
        _bass_rust.insert_act_table_loads(self, tables)

    nc.insert_act_table_loads = _pytypes.MethodType(patched, nc)

P = 128
C = 8192
D = 256
B = 1024
NCORES = 8
CPC = C // NCORES          # classes per core (1024)
NB = CPC // P              # own row blocks (8)
CT = C // P                # class tiles (64)
TEMP = 0.1
BASE_TEMP = 0.1

F32 = mybir.dt.float32
F16 = mybir.dt.float16  # logits matmul operand dtype (loss rel err ~1e-7 vs fp32)
I32 = mybir.dt.int32
I16 = mybir.dt.int16


def _ins(x):
    return getattr(x, "ins", x)


def _chain_structure(labels):
    """Group sample indices by class; lanes sorted by chain length desc."""
    occ = {}
    for t, c in enumerate(labels):
        occ.setdefault(int(c), []).append(t)
    lanes = sorted(occ.items(), key=lambda kv: (-len(kv[1]), kv[0]))
    S = len(lanes)
    R = len(lanes[0][1])
    S_r = [sum(1 for _, ts in lanes if len(ts) > r) for r in range(R)]
    return lanes, S, R, S_r


def build_program(S, R, S_r, NT, NFT, fo_list):
    """One SPMD Bass program; all shape-relevant values are rotation-invariant."""
    nc = bacc.Bacc("TRN2", target_bir_lowering=False, debug=False, num_devices=NCORES)
    _pin_act_tables(nc)
    # ufg = host-pregathered [proto rows of distinct labels; per-round feature
    # rows], lane-major: ufg[p, t, :] is lane p of tile t.  Rotation-invariant.
    ufg_d = nc.declare_dram_parameter("ufg", [P, NT + NFT, D], F32, isOutput=False)
    # proto16 has the to-be-updated rows HOST-ZEROED so dma_scatter_add == assign
    proto16 = nc.declare_dram_parameter("proto16", [C, D], F16, isOutput=False)
    sidx_d = nc.declare_dram_parameter("sidx", [P, NT * P // 16], I16, isOutput=False)
    out_d = nc.declare_dram_parameter("partial", [1, 1], F32, isOutput=True)

    with tile.TileContext(nc) as tc:
        with ExitStack() as ctx:
            aux = ctx.enter_context(tc.tile_pool(name="aux", bufs=1))
            chainp = ctx.enter_context(tc.tile_pool(name="chain", bufs=1))
            psp = ctx.enter_context(tc.tile_pool(name="ps", bufs=2, space="PSUM"))
            bigp = ctx.enter_context(tc.tile_pool(name="big", bufs=1))
            scrp = ctx.enter_context(tc.tile_pool(name="scr", bufs=2))

            # chain-input load issued first so the 2.4MB transfer overlaps init
            ufg = chainp.tile([P, NT + NFT, D], F32)
            nc.sync.dma_start(ufg[:, :, :], ufg_d[:, :, :])

            ident = aux.tile([P, P], F32)
            make_identity(nc, ident[:])
            # fp16 identity + (-BIG)*identity: one extra PE matmul per row block
            # adds -60000 to the diagonal logit before exp -> exp(10*-60000) == 0
            id16 = aux.tile([P, P], F16)
            nc.vector.tensor_copy(id16[:], ident[:])
            negid16 = aux.tile([P, P], F16)
            nc.vector.tensor_scalar_mul(negid16[:], id16[:], -60000.0)
            ones_sb = aux.tile([P, 1], F32)
            nc.vector.memset(ones_sb[:], 1.0)
            # force the (single) activation table set to load while DMAs run
            dummy = aux.tile([1, 1], F32)
            nc.scalar.activation(
                out=dummy[:], in_=ones_sb[0:1, 0:1], func=mybir.ActivationFunctionType.Ln
            )
            dummy2 = aux.tile([1, 1], F32)
            nc.scalar.activation(
                out=dummy2[:], in_=ufg[0:1, 0, 0:1], func=mybir.ActivationFunctionType.Ln
            )

            sidx_sb = aux.tile([P, NT * P // 16], I16)
            nc.sync.dma_start(sidx_sb[:], sidx_d[:])

            # ---- chain compute (replicated) ----
            u = ufg[:, 0:NT, :]
            fg = ufg[:, NT : NT + NFT, :]
            sqd = chainp.tile([P, D], F32)
            n2 = chainp.tile([P, NT], F32)
            lnb = chainp.tile([P, NT], F32)
            rinv = chainp.tile([P, NT], F32)
            nc.vector.memset(n2[:], 1.0)

            # Deferred normalization: track v_{k+1} = v_k + ||v_k|| * f_k (same
            # direction as normalize-each-step since normalize is scale-invariant),
            # then normalize once at the end.  Round 0 has ||v_0|| = 1 exactly.
            fscl = chainp.tile([P, D], F32)
            for r in range(R):
                Sr = S_r[r]
                ntf = Sr // P
                rem = Sr % P
                fo = fo_list[r]
                ntr = ntf + (1 if rem else 0)
                if r == 0:
                    if ntf:
                        nc.vector.tensor_add(
                            u[:, 0:ntf, :], u[:, 0:ntf, :], fg[:, fo : fo + ntf, :]
                        )
                    if rem:
                        nc.vector.tensor_add(
                            u[0:rem, ntf, :], u[0:rem, ntf, :], fg[0:rem, fo + ntf, :]
                        )
                    continue
                for t in range(ntr):
                    pp = P if t < ntf else rem
                    # ||v||^2 via ACT Square + free-axis accumulate (one op,
                    # keeps the DVE free for the mul/add of the next lane tile)
                    nc.scalar.activation(
                        out=sqd[0:pp, :],
                        in_=u[0:pp, t, :],
                        func=mybir.ActivationFunctionType.Square,
                        accum_out=n2[0:pp, t : t + 1],
                    )
                    # ||v|| = exp(0.5*ln(n2)); the Sqrt table is low-precision
                    # (65536-ULP budget) while Ln/Exp are ~2 ULP and share a set
                    nc.scalar.activation(
                        out=lnb[0:pp, t : t + 1],
                        in_=n2[0:pp, t : t + 1],
                        func=mybir.ActivationFunctionType.Ln,
                    )
                    nc.scalar.activation(
                        out=rinv[0:pp, t : t + 1],
                        in_=lnb[0:pp, t : t + 1],
                        func=mybir.ActivationFunctionType.Exp,
                        scale=0.5,
                    )
                    nc.vector.tensor_scalar_mul(
                        fscl[0:pp, :], fg[0:pp, fo + t, :], rinv[0:pp, t : t + 1]
                    )
                    nc.vector.tensor_add(u[0:pp, t, :], u[0:pp, t, :], fscl[0:pp, :])

            # ---- normalize+cast fused, scatter into the fp16 proto copy ----
            # Lanes touched by rounds >=1 all sit in tiles < tb (length-sorted),
            # so tiles tb.. finalize right after round 0, overlapping the rounds.
            tb = 0 if R == 1 else (S_r[1] + P - 1) // P
            u16 = chainp.tile([P, NT, D], F16)

            def finalize(lo, hi):
                if hi <= lo:
                    return
                for t in range(lo, hi):
                    nc.scalar.activation(
                        out=sqd[:],
                        in_=u[:, t, :],
                        func=mybir.ActivationFunctionType.Square,
                        accum_out=n2[:, t : t + 1],
                    )
                nc.scalar.activation(
                    out=lnb[:, lo:hi], in_=n2[:, lo:hi], func=mybir.ActivationFunctionType.Ln
                )
                nc.scalar.activation(
                    out=rinv[:, lo:hi],
                    in_=lnb[:, lo:hi],
                    func=mybir.ActivationFunctionType.Exp,
                    scale=-0.5,
                )
                for t in range(lo, hi):
                    nc.vector.tensor_scalar_mul(
                        u16[:, t, :], u[:, t, :], rinv[:, t : t + 1]
                    )

            finalize(tb, NT)  # overlaps rounds >=1 (emitted above in program order)
            finalize(0, tb)

            # GPSIMD touches pull the sidx-DMA / u16-DVE waits onto their own
            # trivial instructions (one-wait-per-instruction ISA cap), then ONE
            # merged scatter-add covers all tiles (trailing -1 rows skipped;
            # num_idxs_reg counts exactly the real rows)
            gpt_i = chainp.tile([1, 1], I16)
            gpt_h = chainp.tile([1, NT, 1], F16)
            nc.gpsimd.tensor_copy(gpt_i[:], sidx_sb[0:1, 0:1])
            nc.gpsimd.tensor_copy(gpt_h[:], u16[0:1, 0:NT, 0:1])
            scats = [
                nc.gpsimd.dma_scatter_add(
                    out_ap=proto16[:, :],
                    in_ap=u16[:, 0:NT, :],
                    idxs_ap=sidx_sb[:, :],
                    num_idxs=NT * P,
                    num_idxs_reg=S,
                    elem_size=D,
                    single_packet=False,
                )
            ]

            # ---- protoT via xbar DMA transpose (no PE work) ----
            # Sync HWDGE ring only: transposes on nc.scalar would occupy the ACT
            # engine's queue and stall the exp stream.
            ptT = [bigp.tile([P, C], F16, name=f"ptT{h}", tag=f"ptT{h}") for h in range(2)]
            RC = 2048  # row-chunked so the first matmuls can start early
            for rc in range(C // RC):
                for h in range(2):
                    tr = nc.sync.dma_start_transpose(
                        ptT[h][:, rc * RC : (rc + 1) * RC],
                        proto16[rc * RC : (rc + 1) * RC, h * P : (h + 1) * P],
                    )
                    for si in scats:
                        add_dep_helper(
                            _ins(tr),
                            _ins(si),
                            sync=True,
                            reason="transpose after scatter",
                        )

            # ---- own row-block x all-columns matmul + exp row sums ----
            GW = 2048  # psum group width: 4 banks, double-buffered = all 8 banks
            NG = C // GW
            NS = GW // 512
            rs = bigp.tile([P, NB * NG], F32)
            rsum = aux.tile([P, NB], F32)
            mp2 = aux.tile([P, NB], F32)
            # g outer: group g only needs transpose chunk g, so matmuls start
            # as soon as the first chunk lands instead of after all four
            for g in range(NG):
                for b in range(NB):
                    ps = psp.tile([P, GW], F32, tag="ps")
                    for h in range(2):
                        for s in range(NS):
                            nc.tensor.matmul(
                                out=ps[:, s * 512 : (s + 1) * 512],
                                lhsT=ptT[h][:, b * P : (b + 1) * P],
                                rhs=ptT[h][:, g * GW + s * 512 : g * GW + (s + 1) * 512],
                                start=(h == 0),
                                stop=(h == 1) and not (g == 0 and s == b // 4),
                            )
                    if g == 0:
                        # own classes sit at rotated cols 0..CPC; row p of block b is
                        # class b*P+p -> accumulate -60000 onto the exact diagonal
                        # (PE-only masking; exp(10 * (logit - 60000)) == 0)
                        nc.tensor.matmul(
                            out=ps[:, b * P : (b + 1) * P],
                            lhsT=negid16[:],
                            rhs=id16[:],
                            start=False,
                            stop=True,
                        )
                    scr = scrp.tile([P, GW], F32, tag="esc")
                    nc.scalar.activation(
                        out=scr[:],
                        in_=ps[:],
                        func=mybir.ActivationFunctionType.Exp,
                        scale=1.0 / TEMP,
                        accum_out=rs[:, b * NG + g : b * NG + g + 1],
                    )
                    if g == NG - 1:
                        # block b is complete: row sums + log overlap the
                        # remaining blocks' matmuls
                        nc.vector.tensor_reduce(
                            out=rsum[:, b : b + 1],
                            in_=rs[:, b * NG : (b + 1) * NG],
                            axis=mybir.AxisListType.X,
                            op=mybir.AluOpType.add,
                        )
            nc.scalar.activation(
                out=mp2[:, 0:NB],
                in_=rsum[:, 0:NB],
                func=mybir.ActivationFunctionType.Ln,
                scale=1.0 / (C - 1),
            )
            rp = aux.tile([P, 1], F32)
            nc.vector.tensor_reduce(
                out=rp[:], in_=mp2[:], axis=mybir.AxisListType.X, op=mybir.AluOpType.add
            )
            pfin = psp.tile([1, 1], F32, tag="ps")
            nc.tensor.matmul(out=pfin[:], lhsT=rp[:], rhs=ones_sb[:], start=True, stop=True)
            osb = aux.tile([1, 1], F32)
            nc.vector.tensor_copy(osb[:], pfin[:])
            nc.sync.dma_start(out_d[:], osb[:])

    nc.compile()
    return nc


def _host_meta(labels):
    lanes, S, R, S_r = _chain_structure(labels)
    NT = (S + P - 1) // P
    fo_list = []
    off = 0
    for r in range(R):
        fo_list.append(off)
        off += (S_r[r] + P - 1) // P
    NFT = off

    fflat = np.zeros(NFT * P, dtype=np.int64)
    for r in range(R):
        for L in range(S_r[r]):
            fflat[fo_list[r] * P + L] = lanes[L][1][r]
    lane_class = np.array([c for c, _ in lanes], dtype=np.int64)
    return lanes, S, R, S_r, NT, NFT, fo_list, fflat, lane_class


def _wrap_idx16(flat):
    """CounterMachine index layout: flat[i] at [16*rep + i%16, i//16], 8 replicas."""
    n = len(flat)
    assert n % 16 == 0
    blk = flat.reshape(n // 16, 16).T.astype(np.int16)  # [16, n/16]
    return np.tile(blk, (8, 1))  # [128, n/16]


def prepare(features, prototypes, labels):
    """Host-side specialization: build the SPMD program and per-core inputs."""
    features = np.asarray(features, dtype=np.float32)
    prototypes = np.asarray(prototypes, dtype=np.float32)
    labels_np = np.asarray(labels).astype(np.int64)

    lanes, S, R, S_r, NT, NFT, fo_list, fflat, lane_class = _host_meta(labels_np)
    nc = build_program(S, R, S_r, NT, NFT, fo_list)

    # Host-pregathered chain inputs, identical for every core: tile t<NT lane p
    # holds the proto row of distinct label L=t*128+p (pad: proto row 0); tiles
    # >=NT hold per-round feature rows in lane order (pad: feature row 0).
    gflat = np.zeros((NT + NFT) * P, dtype=np.int64)
    gflat[:S] = lane_class
    ufg_rows = np.concatenate(
        [prototypes[gflat[: NT * P]], features[fflat]]
    )  # [(NT+NFT)*128, 256]
    ufg_host = np.ascontiguousarray(
        ufg_rows.reshape(NT + NFT, P, D).transpose(1, 0, 2)
    )  # [P, NT+NFT, D], lane-major

    proto16_full = prototypes.astype(np.float16)
    proto16_full[lane_class] = 0  # scatter-ADD targets must start at zero

    in_maps = []
    for r0 in range(NCORES):
        rot_class = (lane_class - r0 * CPC) % C  # per-core rotated class ids
        sflat = np.full(NT * P, -1, dtype=np.int64)  # -1 = skipped
        sflat[:S] = rot_class
        protoc16 = np.ascontiguousarray(np.roll(proto16_full, -r0 * CPC, axis=0))
        in_maps.append(
            {
                "ufg": ufg_host,
                "proto16": protoc16,
                "sidx": _wrap_idx16(sflat),
            }
        )

    return nc, in_maps


def kernel(features, prototypes, labels):
    nc, in_maps = prepare(features, prototypes, labels)
    res = run_bass_kernel_spmd(nc, in_maps, list(range(NCORES)))
    partials = [float(res.results[i]["partial"][0, 0]) for i in range(NCORES)]
    loss = (TEMP / BASE_TEMP) * (sum(partials) / C)
    return np.asarray(loss, dtype=np.float32)

